# revision 32
# baseline (speedup 1.0000x reference)
"""NVFP4Linear (fused RMSNorm + NVFP4 quant-dequant + GEMM) on 8 TRN2 cores.

Final design (927us HW, rel err 0.0181 vs the 2e-2 gate, deterministic):
  - v2 skeleton: exact e4m3/e2m1 bit-trick quantization, [p,b,s]-contiguous
    quant passes, 2-stage fp8 AllGather of transposed quantized weights,
    512-col matmul chunks.
  - weights quantized straight to fp8: w8 = e4m3(qw*32).  Weight fp4*scale
    products almost never need a 4th mantissa bit (their scales are
    subnormal e4m3), so this costs only 2.5e-3 rel err and halves weight
    bytes end-to-end.  bf16 x fp8 matmuls stream at the same 263ns/512-col
    rate as bf16 x bf16 (PE moving-fetch is byte-rate-bound).
  - mixed-precision k-split (KF0=4): k-tiles [0,4) contract exactly
    (bf16 qx*16 x w8); k-tiles [4,32) use X8 = e4m3(qx*16) in k-paired fp8
    DoubleRow matmuls (256-deep contraction, 2x FLOP rate).  X8 rounding on
    7/8 of the k-sum adds sqrt(0.875)*0.0191 rel err; the qx*16 scaling is
    an exact power of two so both paths share one 1/512 output scale.
    (A full hi/lo-exact fp8 split was measured: DoubleRow with duplicated
    halves is byte-bound at the same speed as bf16 - exactness costs the
    entire 2x, so it was dropped for this calibrated partial split.)
  - X8 transposes ride the PE in fp8 step-2 psum mode (collective-immune);
    the bf16 k-slice uses xbar transposes, PE-mode for the first 4 tiles.
  - quant emitted ahead of transposes, transposes spread between GEMM
    chunks so PE never bunches up behind the DVE.
  - one SBUF tile per weight chunk, loaded with 2 DMAs (SWDGE issue cost
    gated the first matmul after every AllGather / group boundary).
"""

import sys

for _p in ("/opt/trn_rl_repo", "/root/.axon_site/_ro/trn_rl_repo"):
    if _p not in sys.path:
        sys.path.append(_p)

import numpy as np
import concourse.bass as bass  # noqa: F401
import concourse.mybir as mybir
import concourse.tile as tile
from concourse import bacc
from concourse.bass_utils import run_bass_kernel_spmd

dt = mybir.dt
Alu = mybir.AluOpType
Act = mybir.ActivationFunctionType

BLK = 16
EXP_MASK = 0x7F800000
F4_EXP_MIN = 0x3F800000
F4_H_ADD = 0x0B400000
E4M3_EXP_SUB = 0x01800000
E4M3_U_MIN = 0x3B000000
E4M3_H_ADD = 0x0BC00000
C8_MAX = E4M3_U_MIN + E4M3_EXP_SUB  # max first, then add
C8_ADD = E4M3_H_ADD - E4M3_EXP_SUB

N_CORES = 8
M_FULL, K, N = 16384, 4096, 4096
M_SHARD = M_FULL // N_CORES          # 2048
W_SHARD = N // N_CORES               # 512
KT = K // 128                        # 32
KT4 = KT // 4                        # 8 (k-quarter tiles for weight chunks)
KH = K // 2                          # 2048
M_TILES = M_SHARD // 128             # 16
M_GROUP = 4
G = M_TILES // M_GROUP               # 4
HALF_COLS = W_SHARD // 2             # 256
QC = 1024                            # ACT square quarter size
W_PRE = 32.0                         # qw prescale into e4m3 normal range
X_PRE = 16.0                         # qx prescale (exact power of two)
OUT_SCALE = 1.0 / (W_PRE * X_PRE)
KF0 = 4                              # k-tiles [0,KF0): exact bf16 path
KD = (KT - KF0) // 2                 # fp8 DoubleRow k-pairs


def _build():
    nc = bacc.Bacc("TRN2", target_bir_lowering=False, debug=False,
                   num_devices=N_CORES)

    x_sh = nc.declare_dram_parameter("x_sh", [M_SHARD, K], dt.bfloat16,
                                     isOutput=False)
    w_sh = nc.declare_dram_parameter("w_sh", [W_SHARD, K], dt.bfloat16,
                                     isOutput=False)
    # output stored bf16 on-device (halves store traffic); host widens to
    # fp32 — adds ~3e-4 RMS rounding, far inside the 2e-2 gate
    out = nc.declare_dram_parameter("out", [M_SHARD, N], dt.bfloat16,
                                    isOutput=True)

    # quantized transposed weight, layout (kp, kc, n):
    #   element = qw^T[k = kc*128+kp, shard col n]
    wq_locA = nc.dram_tensor("wq_locA", [128, KT * HALF_COLS], dt.float8e4)
    wq_locB = nc.dram_tensor("wq_locB", [128, KT * HALF_COLS], dt.float8e4)
    wq_fullA = nc.dram_tensor("wq_fullA", [N_CORES * 128, KT * HALF_COLS],
                              dt.float8e4, addr_space="Shared")
    wq_fullB = nc.dram_tensor("wq_fullB", [N_CORES * 128, KT * HALF_COLS],
                              dt.float8e4, addr_space="Shared")

    with tile.TileContext(nc) as tc:
        with (
            tc.tile_pool(name="src", bufs=4) as p_src,
            tc.tile_pool(name="f32", bufs=2) as p_f32,
            tc.tile_pool(name="q", bufs=4) as p_q,
            tc.tile_pool(name="q8w", bufs=2) as p_q8w,
            tc.tile_pool(name="wt8", bufs=2) as p_wt8,
            tc.tile_pool(name="sm", bufs=2) as p_sm,
            tc.tile_pool(name="row", bufs=4) as p_row,
            tc.tile_pool(name="qxT", bufs=2 * M_GROUP) as p_qxT,
            tc.tile_pool(name="qT28", bufs=2) as p_qT28,
            tc.tile_pool(name="x8T", bufs=2 * M_GROUP) as p_x8T,
            tc.tile_pool(name="qwT", bufs=2) as p_qwT,
            tc.tile_pool(name="stage", bufs=2) as p_stage,
            tc.tile_pool(name="psum", bufs=4, space="PSUM") as p_psum,
            tc.tile_pool(name="pst", bufs=1, space="PSUM") as p_pst,
            tc.tile_pool(name="pstw", bufs=2, space="PSUM") as p_pstw,
            tc.tile_pool(name="sqp", bufs=1, space="PSUM") as p_sqp,
        ):
            def quant_tile(dram_src, row0, with_rms, with_clamp):
                """Quantize 128 rows; returns q tile [128, K] bf16."""
                srcs = []
                for hi in range(2):
                    s = p_src.tile([128, KH], dt.bfloat16, tag="src")
                    nc.sync.dma_start(
                        out=s[:],
                        in_=dram_src[row0:row0 + 128, hi * KH:(hi + 1) * KH])
                    srcs.append(s)

                inv_rms_ap = None
                if with_rms:
                    ssum = p_row.tile([128, 8], dt.float32, tag="ssum")
                    for qi in range(8):
                        sq = p_sqp.tile([128, 512], dt.float32, tag="sq")
                        sh = srcs[qi // 4]
                        o = (qi % 4) * 512
                        nc.scalar.activation(
                            out=sq[:], in_=sh[:, o:o + 512],
                            func=Act.Square, accum_out=ssum[:, qi:qi + 1])
                    ssum2 = p_row.tile([128, 1], dt.float32, tag="ssum2")
                    nc.vector.tensor_reduce(
                        out=ssum2[:], in_=ssum[:],
                        axis=mybir.AxisListType.X, op=Alu.add)
                    ms = p_row.tile([128, 1], dt.float32, tag="ms")
                    nc.vector.tensor_scalar(
                        out=ms[:], in0=ssum2[:],
                        scalar1=float(np.float32(1.0 / K)), scalar2=1e-6,
                        op0=Alu.mult, op1=Alu.add)
                    srms = p_row.tile([128, 1], dt.float32, tag="srms")
                    nc.scalar.activation(out=srms[:], in_=ms[:],
                                         func=Act.Sqrt)
                    invr = p_row.tile([128, 1], dt.float32, tag="invr")
                    nc.vector.reciprocal(invr[:], srms[:])
                    inv_rms_ap = invr[:]

                # ---- per-block scales (tile-wide, [128, 256]) ----
                nb = K // BLK
                nbh = nb // 2
                amax = p_sm.tile([128, nb], dt.float32, tag="amax")
                for hi in range(2):
                    nc.vector.tensor_reduce(
                        out=amax[:, hi * nbh:(hi + 1) * nbh],
                        in_=srcs[hi][:].rearrange("p (b s) -> p b s", s=BLK),
                        axis=mybir.AxisListType.X,
                        op=Alu.max, apply_absolute_value=True)

                v = p_sm.tile([128, nb], dt.float32, tag="v")
                if inv_rms_ap is not None:
                    nc.vector.tensor_scalar(
                        out=v[:], in0=amax[:], scalar1=inv_rms_ap,
                        scalar2=float(np.float32(1.0 / 6.0)),
                        op0=Alu.mult, op1=Alu.mult)
                else:
                    nc.vector.tensor_scalar(
                        out=v[:], in0=amax[:],
                        scalar1=float(np.float32(1.0 / 6.0)), scalar2=None,
                        op0=Alu.mult)

                h8 = p_sm.tile([128, nb], dt.float32, tag="h8")
                nc.vector.tensor_scalar(
                    out=h8[:].bitcast(dt.int32), in0=v[:].bitcast(dt.int32),
                    scalar1=EXP_MASK, scalar2=None, op0=Alu.bitwise_and)
                nc.vector.tensor_scalar(
                    out=h8[:].bitcast(dt.int32), in0=h8[:].bitcast(dt.int32),
                    scalar1=C8_MAX, scalar2=C8_ADD, op0=Alu.max, op1=Alu.add)
                scal = p_sm.tile([128, nb], dt.float32, tag="scal")
                nc.vector.tensor_tensor(out=scal[:], in0=v[:], in1=h8[:],
                                        op=Alu.add)
                nc.vector.tensor_tensor(out=scal[:], in0=scal[:], in1=h8[:],
                                        op=Alu.subtract)
                g = p_sm.tile([128, nb], dt.float32, tag="g")
                nc.vector.reciprocal(g[:], scal[:])
                if inv_rms_ap is not None:
                    nc.vector.tensor_scalar(
                        out=g[:], in0=g[:], scalar1=inv_rms_ap,
                        scalar2=1.0e30, op0=Alu.mult, op1=Alu.min)
                else:
                    # weight path: fold ties-up fudge (1+2^-23) into g
                    nc.vector.tensor_scalar(
                        out=g[:], in0=g[:],
                        scalar1=float(np.float32(1.0 + 2.0 ** -23)),
                        scalar2=1.0e30, op0=Alu.mult, op1=Alu.min)

                if with_rms:
                    q = p_q.tile([128, K], dt.bfloat16, tag="q")
                else:
                    q = p_q8w.tile([128, K], dt.float8e4, tag="q8w")

                # ---- big passes, per half ----
                for hi in range(2):
                    bsl = slice(hi * nbh, (hi + 1) * nbh)
                    ax = p_f32.tile([128, KH], dt.float32, tag="ax")
                    hc = p_f32.tile([128, KH], dt.float32, tag="hc")
                    ax_bs = ax[:].rearrange("p (b s) -> p b s", s=BLK)
                    g_b = g[:, bsl, None].broadcast_to([128, nbh, BLK])
                    src_h = srcs[hi][:].rearrange("p (b s) -> p b s", s=BLK)
                    # P1: ax = src * g
                    nc.vector.tensor_tensor(out=ax_bs, in0=src_h, in1=g_b,
                                            op=Alu.mult)
                    if with_clamp:
                        nc.vector.tensor_scalar(
                            out=ax[:], in0=ax[:], scalar1=6.0, scalar2=-6.0,
                            op0=Alu.min, op1=Alu.max)
                    # P2: hc = ax & EXP_MASK
                    nc.vector.tensor_scalar(
                        out=hc[:].bitcast(dt.int32),
                        in0=ax[:].bitcast(dt.int32),
                        scalar1=EXP_MASK, scalar2=None, op0=Alu.bitwise_and)
                    # P3: hc = (hc max F4_EXP_MIN) + F4_H_ADD
                    nc.vector.tensor_scalar(
                        out=hc[:].bitcast(dt.int32),
                        in0=hc[:].bitcast(dt.int32),
                        scalar1=F4_EXP_MIN, scalar2=F4_H_ADD,
                        op0=Alu.max, op1=Alu.add)
                    # P4/P5: ax = (ax + hc) - hc = round(ax)  (in-place, 2x)
                    nc.vector.tensor_tensor(out=ax[:], in0=ax[:], in1=hc[:],
                                            op=Alu.add)
                    nc.vector.tensor_tensor(out=ax[:], in0=ax[:], in1=hc[:],
                                            op=Alu.subtract)
                    # P6: q = ax * scal  (x: bf16; w: e4m3(q * 32))
                    q_bs = q[:, hi * KH:(hi + 1) * KH].rearrange(
                        "p (b s) -> p b s", s=BLK)
                    scal_b = scal[:, bsl, None].broadcast_to([128, nbh, BLK])
                    pre = X_PRE if with_rms else W_PRE
                    nc.vector.scalar_tensor_tensor(
                        out=q_bs, in0=ax_bs, scalar=pre, in1=scal_b,
                        op0=Alu.mult, op1=Alu.mult)
                return q

            def x8T_from_qT(qT28):
                """X8 = e4m3(qx*16) derived AFTER the transpose: one ACT
                convert of the transposed bf16 planes lands directly in the
                canonical DoubleRow [kp, pair, half, m] fp8 layout."""
                x8T = p_x8T.tile([128, KD, 2, 128], dt.float8e4, tag="x8T")
                nc.scalar.activation(
                    out=x8T[:].rearrange("p j t m -> p (j t) m"),
                    in_=qT28[:], func=Act.Copy)
                return x8T

            def transpose_q(q):
                """xbar transposes (off the PE; stall while a collective is
                in flight) + fp8 convert of the DoubleRow k-slice."""
                t = p_qxT.tile([128, KF0, 128], dt.bfloat16, tag="qxT")
                nc.sync.dma_start_transpose(out=t[:], in_=q[:, 0:KF0 * 128])
                qT28 = p_qT28.tile([128, KT - KF0, 128], dt.bfloat16,
                                   tag="qT28")
                nc.sync.dma_start_transpose(out=qT28[:],
                                            in_=q[:, KF0 * 128:K])
                return t, x8T_from_qT(qT28)

            # identity for PE-mode transpose (collective-immune)
            ones = p_row.tile([128, 128], dt.bfloat16, tag="ones")
            nc.vector.memset(ones[:], 1.0)
            ident = p_row.tile([128, 128], dt.bfloat16, tag="ident")
            nc.gpsimd.affine_select(
                out=ident[:], in_=ones[:], pattern=[[-1, 128]],
                compare_op=Alu.is_equal, fill=0.0,
                base=0, channel_multiplier=1)

            def transpose_q_pe(q):
                """Transpose on the (idle) PE + ACT copies, no xbar
                (collective-immune, for group-0 tiles)."""
                t = p_qxT.tile([128, KF0, 128], dt.bfloat16, tag="qxT")
                ps = p_pst.tile([128, 4, 128], dt.bfloat16, tag="pst")
                for j in range(KF0):
                    nc.tensor.transpose(
                        out=ps[:, j, :], in_=q[:, j * 128:(j + 1) * 128],
                        identity=ident[:])
                nc.scalar.copy(t[:], ps[:, 0:KF0, :])
                qT28 = p_qT28.tile([128, KT - KF0, 128], dt.bfloat16,
                                   tag="qT28")
                for kb in range((KT - KF0) // 4):
                    ps = p_pst.tile([128, 4, 128], dt.bfloat16, tag="pst")
                    for j in range(4):
                        kc = KF0 + kb * 4 + j
                        nc.tensor.transpose(
                            out=ps[:, j, :],
                            in_=q[:, kc * 128:(kc + 1) * 128],
                            identity=ident[:])
                    nc.scalar.copy(qT28[:, kb * 4:(kb + 1) * 4, :], ps[:])
                return t, x8T_from_qT(qT28)

            ones8 = p_row.tile([128, 128], dt.float8e4, tag="ones8")
            nc.vector.memset(ones8[:], 1.0)
            ident8 = p_row.tile([128, 128], dt.float8e4, tag="ident8")
            nc.gpsimd.affine_select(
                out=ident8[:], in_=ones8[:], pattern=[[-1, 128]],
                compare_op=Alu.is_equal, fill=0.0,
                base=0, channel_multiplier=1)

            def transpose_w_pe(q8):
                """PE fp8 transpose (step-2 psum) -> [kp, kc, n] fp8."""
                t = p_wt8.tile([128, KT, 128], dt.float8e4, tag="wt8")
                for kb in range(KT // 4):
                    psD = p_pstw.tile([128, 8, 128, 2], dt.float8e4,
                                      tag="pstw")
                    ps = psD[:, 0:4]
                    for j in range(4):
                        kc = kb * 4 + j
                        nc.tensor.transpose(
                            out=ps[:, j, :, 0],
                            in_=q8[:, kc * 128:(kc + 1) * 128],
                            identity=ident8[:])
                    nc.scalar.copy(t[:, kb * 4:(kb + 1) * 4, :],
                                   ps[:, :, :, 0])
                return t

            # ---------- prologue: weight quant + staged AllGather ----------
            wlA = wq_locA.ap().rearrange("p (kc n) -> p kc n", n=HALF_COLS)
            wlB = wq_locB.ap().rearrange("p (kc n) -> p kc n", n=HALF_COLS)
            def w_tile(wt):
                qw = quant_tile(w_sh, wt * 128, with_rms=False,
                                with_clamp=True)
                t = transpose_w_pe(qw)
                dst = wlA if wt < 2 else wlB
                c0 = (wt % 2) * 128
                nc.sync.dma_start(out=dst[:, :, c0:c0 + 128], in_=t[:])

            def fire_ag(loc, full):
                nc.gpsimd.collective_compute(
                    "AllGather", Alu.bypass,
                    replica_groups=[list(range(N_CORES))],
                    ins=[loc.ap().opt()], outs=[full.ap().opt()])

            # ---------- x quant ----------
            qxT = {}

            def quant_group(gi):
                for mt in range(gi * M_GROUP, (gi + 1) * M_GROUP):
                    q = quant_tile(x_sh, mt * 128, with_rms=True,
                                   with_clamp=False)
                    qxT[mt] = transpose_q(q)

            # ---------- GEMM ----------
            wfA = wq_fullA.ap().rearrange("(c p) (kc n) -> c p kc n",
                                          p=128, n=HALF_COLS)
            wfB = wq_fullB.ap().rearrange("(c p) (kc n) -> c p kc n",
                                          p=128, n=HALF_COLS)

            def chunk_defs():
                """(stage_aps, out_cols): two 256-col strips per chunk."""
                for j in range(4):
                    ca, cb = 2 * j, 2 * j + 1
                    yield ([wfA[ca], wfA[cb]],
                           [W_SHARD * ca, W_SHARD * cb])
                for j in range(4):
                    ca, cb = 2 * j, 2 * j + 1
                    yield ([wfB[ca], wfB[cb]],
                           [W_SHARD * ca + HALF_COLS,
                            W_SHARD * cb + HALF_COLS])

            def load_chunk(srcs_ap, eng=None):
                """One full-K tile per chunk, loaded with 2 DMAs: SWDGE
                issue cost (8 DMAs before) gated the first matmul after
                every AllGather / group boundary."""
                eng = eng or nc.gpsimd
                wt = p_qwT.tile([128, KT, W_SHARD], dt.float8e4, tag="qwT")
                eng.dma_start(out=wt[:, :, 0:HALF_COLS], in_=srcs_ap[0])
                eng.dma_start(out=wt[:, :, HALF_COLS:W_SHARD],
                              in_=srcs_ap[1])
                return wt

            def gemm_m(wk, out_cols, mt):
                ps = p_psum.tile([128, W_SHARD], dt.float32, tag="ps")
                tb, t8 = qxT[mt]
                for k in range(KF0):
                    nc.tensor.matmul(
                        ps[:], lhsT=tb[:, k, :], rhs=wk[:, k, :],
                        start=(k == 0), stop=False)
                for j in range(KD):
                    k = KF0 + 2 * j
                    nc.tensor.matmul(
                        ps[:], lhsT=t8[:, j, :, :], rhs=wk[:, k:k + 2, :],
                        start=False, stop=(j == KD - 1),
                        perf_mode=mybir.MatmulPerfMode.DoubleRow)
                st = p_stage.tile([128, W_SHARD], dt.bfloat16, tag="st")
                nc.scalar.activation(out=st[:], in_=ps[:], func=Act.Copy,
                                     scale=float(np.float32(OUT_SCALE)))
                for si, dst_col in enumerate(out_cols):
                    nc.sync.dma_start(
                        out=out[mt * 128:(mt + 1) * 128,
                                dst_col:dst_col + HALF_COLS],
                        in_=st[:, si * HALF_COLS:(si + 1) * HALF_COLS])

            def gemm_group(gi):
                for srcs_ap, out_cols in chunk_defs():
                    wk = load_chunk(srcs_ap, eng=nc.sync)
                    for mt in range(gi * M_GROUP, (gi + 1) * M_GROUP):
                        gemm_m(wk, out_cols, mt)

            # ----- group 0: interleave x-quant (PE transposes) with the
            # first chunk's per-m matmuls so the PE starts as soon as
            # AG-A lands and x0 is quantized -----
            chunks = list(chunk_defs())
            w_tile(0)
            w_tile(1)
            fire_ag(wq_locA, wq_fullA)
            w_tile(2)
            w_tile(3)
            fire_ag(wq_locB, wq_fullB)
            for mt in (0, 1):
                qm = quant_tile(x_sh, mt * 128, with_rms=True,
                                with_clamp=False)
                qxT[mt] = transpose_q_pe(qm)
            # A-phase of group 0, m-pair-major: each A-chunk is loaded,
            # used for an m-pair, reloaded for the next pair; B-chunk
            # loads ride the gpsimd (SWDGE) queue so their AG-B wait
            # cannot block the sync queue.
            for rep in range(2):
                if rep == 1:
                    for mt in (2, 3):
                        qm = quant_tile(x_sh, mt * 128, with_rms=True,
                                        with_clamp=False)
                        qxT[mt] = transpose_q_pe(qm)
                for src_ap, out_cols in chunks[0:4]:
                    wk = load_chunk(src_ap)
                    for mt in (2 * rep, 2 * rep + 1):
                        gemm_m(wk, out_cols, mt)
            pend = {}

            def quant_only(mts):
                for mt in mts:
                    pend[mt] = quant_tile(x_sh, mt * 128, with_rms=True,
                                          with_clamp=False)

            def transpose_mt(mt):
                qxT[mt] = transpose_q(pend.pop(mt))

            def gemm_group2(gi, next_mts):
                nx = list(next_mts)
                for ci, (src_ap, out_cols) in enumerate(chunk_defs()):
                    wk = load_chunk(src_ap, eng=nc.sync)
                    for mt in range(gi * M_GROUP, (gi + 1) * M_GROUP):
                        gemm_m(wk, out_cols, mt)
                    if ci % 2 == 0 and nx:
                        transpose_mt(nx.pop(0))

            quant_only([4, 5, 6, 7])
            for ci, (src_ap, out_cols) in enumerate(chunks[4:]):
                wk = load_chunk(src_ap, eng=nc.gpsimd)
                for mt in range(4):
                    gemm_m(wk, out_cols, mt)
                transpose_mt(4 + ci)
            quant_only([8, 9, 10, 11])
            gemm_group2(1, [8, 9, 10, 11])
            quant_only([12, 13, 14, 15])
            gemm_group2(2, [12, 13, 14, 15])
            gemm_group2(3, [])

    nc.compile()
    return nc


_NC = None


def kernel(x, weight):
    global _NC
    if _NC is None:
        _NC = _build()
    x = np.ascontiguousarray(x)
    weight = np.ascontiguousarray(weight)
    in_maps = [
        {"x_sh": x[c * M_SHARD:(c + 1) * M_SHARD],
         "w_sh": weight[c * W_SHARD:(c + 1) * W_SHARD]}
        for c in range(N_CORES)
    ]
    res = run_bass_kernel_spmd(_NC, in_maps, list(range(N_CORES)))
    return np.concatenate(
        [np.asarray(res.results[c]["out"]).astype(np.float32)
         for c in range(N_CORES)], axis=0)



# revision 33
# speedup vs baseline: 1.0576x; 1.0576x over previous
"""NVFP4Linear (fused RMSNorm + NVFP4 quant-dequant + GEMM) on 8 TRN2 cores.

Final design (927us HW, rel err 0.0181 vs the 2e-2 gate, deterministic):
  - v2 skeleton: exact e4m3/e2m1 bit-trick quantization, [p,b,s]-contiguous
    quant passes, 2-stage fp8 AllGather of transposed quantized weights,
    512-col matmul chunks.
  - weights quantized straight to fp8: w8 = e4m3(qw*32).  Weight fp4*scale
    products almost never need a 4th mantissa bit (their scales are
    subnormal e4m3), so this costs only 2.5e-3 rel err and halves weight
    bytes end-to-end.  bf16 x fp8 matmuls stream at the same 263ns/512-col
    rate as bf16 x bf16 (PE moving-fetch is byte-rate-bound).
  - mixed-precision k-split (KF0=4): k-tiles [0,4) contract exactly
    (bf16 qx*16 x w8); k-tiles [4,32) use X8 = e4m3(qx*16) in k-paired fp8
    DoubleRow matmuls (256-deep contraction, 2x FLOP rate).  X8 rounding on
    7/8 of the k-sum adds sqrt(0.875)*0.0191 rel err; the qx*16 scaling is
    an exact power of two so both paths share one 1/512 output scale.
    (A full hi/lo-exact fp8 split was measured: DoubleRow with duplicated
    halves is byte-bound at the same speed as bf16 - exactness costs the
    entire 2x, so it was dropped for this calibrated partial split.)
  - X8 transposes ride the PE in fp8 step-2 psum mode (collective-immune);
    the bf16 k-slice uses xbar transposes, PE-mode for the first 4 tiles.
  - quant emitted ahead of transposes, transposes spread between GEMM
    chunks so PE never bunches up behind the DVE.
  - one SBUF tile per weight chunk, loaded with 2 DMAs (SWDGE issue cost
    gated the first matmul after every AllGather / group boundary).
"""

import sys

for _p in ("/opt/trn_rl_repo", "/root/.axon_site/_ro/trn_rl_repo"):
    if _p not in sys.path:
        sys.path.append(_p)

import numpy as np
import concourse.bass as bass  # noqa: F401
import concourse.mybir as mybir
import concourse.tile as tile
from concourse import bacc
from concourse.bass_utils import run_bass_kernel_spmd

dt = mybir.dt
Alu = mybir.AluOpType
Act = mybir.ActivationFunctionType

BLK = 16
EXP_MASK = 0x7F800000
F4_EXP_MIN = 0x3F800000
F4_H_ADD = 0x0B400000
E4M3_EXP_SUB = 0x01800000
E4M3_U_MIN = 0x3B000000
E4M3_H_ADD = 0x0BC00000
C8_MAX = E4M3_U_MIN + E4M3_EXP_SUB  # max first, then add
C8_ADD = E4M3_H_ADD - E4M3_EXP_SUB

N_CORES = 8
M_FULL, K, N = 16384, 4096, 4096
M_SHARD = M_FULL // N_CORES          # 2048
W_SHARD = N // N_CORES               # 512
KT = K // 128                        # 32
KT4 = KT // 4                        # 8 (k-quarter tiles for weight chunks)
KH = K // 2                          # 2048
M_TILES = M_SHARD // 128             # 16
M_GROUP = 4
G = M_TILES // M_GROUP               # 4
HALF_COLS = W_SHARD // 2             # 256
QC = 1024                            # ACT square quarter size
W_PRE = 32.0                         # qw prescale into e4m3 normal range
X_PRE = 16.0                         # qx prescale (exact power of two)
OUT_SCALE = 1.0 / (W_PRE * X_PRE)
KF0 = 4                              # k-tiles [0,KF0): exact bf16 path
KD = (KT - KF0) // 2                 # fp8 DoubleRow k-pairs


def _build():
    nc = bacc.Bacc("TRN2", target_bir_lowering=False, debug=False,
                   num_devices=N_CORES)

    x_sh = nc.declare_dram_parameter("x_sh", [M_SHARD, K], dt.bfloat16,
                                     isOutput=False)
    w_sh = nc.declare_dram_parameter("w_sh", [W_SHARD, K], dt.bfloat16,
                                     isOutput=False)
    # output stored bf16 on-device (halves store traffic); host widens to
    # fp32 — adds ~3e-4 RMS rounding, far inside the 2e-2 gate
    out = nc.declare_dram_parameter("out", [M_SHARD, N], dt.bfloat16,
                                    isOutput=True)

    # quantized transposed weight, layout (kp, kc, n):
    #   element = qw^T[k = kc*128+kp, shard col n]
    wq_locA = nc.dram_tensor("wq_locA", [128, KT * HALF_COLS], dt.float8e4)
    wq_locB = nc.dram_tensor("wq_locB", [128, KT * HALF_COLS], dt.float8e4)
    wq_fullA = nc.dram_tensor("wq_fullA", [N_CORES * 128, KT * HALF_COLS],
                              dt.float8e4, addr_space="Shared")
    wq_fullB = nc.dram_tensor("wq_fullB", [N_CORES * 128, KT * HALF_COLS],
                              dt.float8e4, addr_space="Shared")

    with tile.TileContext(nc) as tc:
        with (
            tc.tile_pool(name="src", bufs=4) as p_src,
            tc.tile_pool(name="f32", bufs=2) as p_f32,
            tc.tile_pool(name="q", bufs=4) as p_q,
            tc.tile_pool(name="q8w", bufs=2) as p_q8w,
            tc.tile_pool(name="wt8", bufs=2) as p_wt8,
            tc.tile_pool(name="sm", bufs=2) as p_sm,
            tc.tile_pool(name="row", bufs=4) as p_row,
            tc.tile_pool(name="qxT", bufs=2 * M_GROUP) as p_qxT,
            tc.tile_pool(name="x8", bufs=5) as p_x8,
            tc.tile_pool(name="x8T", bufs=2 * M_GROUP) as p_x8T,
            tc.tile_pool(name="qwT", bufs=2) as p_qwT,
            tc.tile_pool(name="stage", bufs=2) as p_stage,
            tc.tile_pool(name="psum", bufs=4, space="PSUM") as p_psum,
            tc.tile_pool(name="pst", bufs=1, space="PSUM") as p_pst,
            tc.tile_pool(name="pstw", bufs=2, space="PSUM") as p_pstw,
            tc.tile_pool(name="sqp", bufs=1, space="PSUM") as p_sqp,
        ):
            def quant_tile(dram_src, row0, with_rms, with_clamp):
                """Quantize 128 rows; returns q tile [128, K] bf16."""
                srcs = []
                for hi in range(2):
                    s = p_src.tile([128, KH], dt.bfloat16, tag="src")
                    nc.sync.dma_start(
                        out=s[:],
                        in_=dram_src[row0:row0 + 128, hi * KH:(hi + 1) * KH])
                    srcs.append(s)

                inv_rms_ap = None
                if with_rms:
                    ssum = p_row.tile([128, 8], dt.float32, tag="ssum")
                    for qi in range(8):
                        sq = p_sqp.tile([128, 512], dt.float32, tag="sq")
                        sh = srcs[qi // 4]
                        o = (qi % 4) * 512
                        nc.scalar.activation(
                            out=sq[:], in_=sh[:, o:o + 512],
                            func=Act.Square, accum_out=ssum[:, qi:qi + 1])
                    ssum2 = p_row.tile([128, 1], dt.float32, tag="ssum2")
                    nc.vector.tensor_reduce(
                        out=ssum2[:], in_=ssum[:],
                        axis=mybir.AxisListType.X, op=Alu.add)
                    ms = p_row.tile([128, 1], dt.float32, tag="ms")
                    nc.vector.tensor_scalar(
                        out=ms[:], in0=ssum2[:],
                        scalar1=float(np.float32(1.0 / K)), scalar2=1e-6,
                        op0=Alu.mult, op1=Alu.add)
                    srms = p_row.tile([128, 1], dt.float32, tag="srms")
                    nc.scalar.activation(out=srms[:], in_=ms[:],
                                         func=Act.Sqrt)
                    invr = p_row.tile([128, 1], dt.float32, tag="invr")
                    nc.vector.reciprocal(invr[:], srms[:])
                    inv_rms_ap = invr[:]

                # ---- per-block scales (tile-wide, [128, 256]) ----
                nb = K // BLK
                nbh = nb // 2
                amax = p_sm.tile([128, nb], dt.float32, tag="amax")
                for hi in range(2):
                    nc.vector.tensor_reduce(
                        out=amax[:, hi * nbh:(hi + 1) * nbh],
                        in_=srcs[hi][:].rearrange("p (b s) -> p b s", s=BLK),
                        axis=mybir.AxisListType.X,
                        op=Alu.max, apply_absolute_value=True)

                v = p_sm.tile([128, nb], dt.float32, tag="v")
                if inv_rms_ap is not None:
                    nc.vector.tensor_scalar(
                        out=v[:], in0=amax[:], scalar1=inv_rms_ap,
                        scalar2=float(np.float32(1.0 / 6.0)),
                        op0=Alu.mult, op1=Alu.mult)
                else:
                    nc.vector.tensor_scalar(
                        out=v[:], in0=amax[:],
                        scalar1=float(np.float32(1.0 / 6.0)), scalar2=None,
                        op0=Alu.mult)

                h8 = p_sm.tile([128, nb], dt.float32, tag="h8")
                nc.vector.tensor_scalar(
                    out=h8[:].bitcast(dt.int32), in0=v[:].bitcast(dt.int32),
                    scalar1=EXP_MASK, scalar2=None, op0=Alu.bitwise_and)
                nc.vector.tensor_scalar(
                    out=h8[:].bitcast(dt.int32), in0=h8[:].bitcast(dt.int32),
                    scalar1=C8_MAX, scalar2=C8_ADD, op0=Alu.max, op1=Alu.add)
                scal = p_sm.tile([128, nb], dt.float32, tag="scal")
                nc.vector.tensor_tensor(out=scal[:], in0=v[:], in1=h8[:],
                                        op=Alu.add)
                nc.vector.tensor_tensor(out=scal[:], in0=scal[:], in1=h8[:],
                                        op=Alu.subtract)
                g = p_sm.tile([128, nb], dt.float32, tag="g")
                nc.vector.reciprocal(g[:], scal[:])
                if inv_rms_ap is not None:
                    nc.vector.tensor_scalar(
                        out=g[:], in0=g[:], scalar1=inv_rms_ap,
                        scalar2=1.0e30, op0=Alu.mult, op1=Alu.min)
                else:
                    # weight path: fold ties-up fudge (1+2^-23) into g
                    nc.vector.tensor_scalar(
                        out=g[:], in0=g[:],
                        scalar1=float(np.float32(1.0 + 2.0 ** -23)),
                        scalar2=1.0e30, op0=Alu.mult, op1=Alu.min)

                if with_rms:
                    q = p_q.tile([128, K], dt.bfloat16, tag="q")
                else:
                    q = p_q8w.tile([128, K], dt.float8e4, tag="q8w")

                # ---- big passes, per half ----
                for hi in range(2):
                    bsl = slice(hi * nbh, (hi + 1) * nbh)
                    ax = p_f32.tile([128, KH], dt.float32, tag="ax")
                    hc = p_f32.tile([128, KH], dt.float32, tag="hc")
                    ax_bs = ax[:].rearrange("p (b s) -> p b s", s=BLK)
                    g_b = g[:, bsl, None].broadcast_to([128, nbh, BLK])
                    src_h = srcs[hi][:].rearrange("p (b s) -> p b s", s=BLK)
                    # P1: ax = src * g
                    nc.vector.tensor_tensor(out=ax_bs, in0=src_h, in1=g_b,
                                            op=Alu.mult)
                    if with_clamp:
                        nc.vector.tensor_scalar(
                            out=ax[:], in0=ax[:], scalar1=6.0, scalar2=-6.0,
                            op0=Alu.min, op1=Alu.max)
                    # P2: hc = ax & EXP_MASK
                    nc.vector.tensor_scalar(
                        out=hc[:].bitcast(dt.int32),
                        in0=ax[:].bitcast(dt.int32),
                        scalar1=EXP_MASK, scalar2=None, op0=Alu.bitwise_and)
                    # P3: hc = (hc max F4_EXP_MIN) + F4_H_ADD
                    nc.vector.tensor_scalar(
                        out=hc[:].bitcast(dt.int32),
                        in0=hc[:].bitcast(dt.int32),
                        scalar1=F4_EXP_MIN, scalar2=F4_H_ADD,
                        op0=Alu.max, op1=Alu.add)
                    # P4/P5: ax = (ax + hc) - hc = round(ax)  (in-place, 2x)
                    nc.vector.tensor_tensor(out=ax[:], in0=ax[:], in1=hc[:],
                                            op=Alu.add)
                    nc.vector.tensor_tensor(out=ax[:], in0=ax[:], in1=hc[:],
                                            op=Alu.subtract)
                    # P6: q = ax * scal  (x: bf16; w: e4m3(q * 32))
                    q_bs = q[:, hi * KH:(hi + 1) * KH].rearrange(
                        "p (b s) -> p b s", s=BLK)
                    scal_b = scal[:, bsl, None].broadcast_to([128, nbh, BLK])
                    pre = X_PRE if with_rms else W_PRE
                    nc.vector.scalar_tensor_tensor(
                        out=q_bs, in0=ax_bs, scalar=pre, in1=scal_b,
                        op0=Alu.mult, op1=Alu.mult)
                if not with_rms:
                    return q
                # X8 = e4m3(qx*16) for the DoubleRow k-half (RTNE; q holds
                # qx*16 exactly in bf16, so this matches the f32 rounding)
                x8 = p_x8.tile([128, K - KF0 * 128], dt.float8e4, tag="x8")
                nc.scalar.activation(out=x8[:], in_=q[:, KF0 * 128:K],
                                     func=Act.Copy)
                return q, x8

            def transpose_q(q):
                """xbar transpose of the exact-bf16 k-half — stalls while a
                collective is in flight."""
                t = p_qxT.tile([128, KF0, 128], dt.bfloat16, tag="qxT")
                nc.sync.dma_start_transpose(out=t[:], in_=q[:, 0:KF0 * 128])
                return t

            # identity for PE-mode transpose (collective-immune)
            ones = p_row.tile([128, 128], dt.bfloat16, tag="ones")
            nc.vector.memset(ones[:], 1.0)
            ident = p_row.tile([128, 128], dt.bfloat16, tag="ident")
            nc.gpsimd.affine_select(
                out=ident[:], in_=ones[:], pattern=[[-1, 128]],
                compare_op=Alu.is_equal, fill=0.0,
                base=0, channel_multiplier=1)

            def transpose_q_pe(q):
                """Transpose on the (idle) PE + ACT copies, no xbar.
                4 k-slices batched per PSUM bank -> 8 ACT copies."""
                t = p_qxT.tile([128, KF0, 128], dt.bfloat16, tag="qxT")
                for kb in range(KF0 // 4):
                    ps = p_pst.tile([128, 4, 128], dt.bfloat16, tag="pst")
                    for j in range(4):
                        kc = kb * 4 + j
                        nc.tensor.transpose(
                            out=ps[:, j, :],
                            in_=q[:, kc * 128:(kc + 1) * 128],
                            identity=ident[:])
                    nc.scalar.copy(t[:, kb * 4:(kb + 1) * 4, :], ps[:])
                return t

            ones8 = p_row.tile([128, 128], dt.float8e4, tag="ones8")
            nc.vector.memset(ones8[:], 1.0)
            ident8 = p_row.tile([128, 128], dt.float8e4, tag="ident8")
            nc.gpsimd.affine_select(
                out=ident8[:], in_=ones8[:], pattern=[[-1, 128]],
                compare_op=Alu.is_equal, fill=0.0,
                base=0, channel_multiplier=1)

            def transpose_w_pe(q8):
                """PE fp8 transpose (step-2 psum) -> [kp, kc, n] fp8."""
                t = p_wt8.tile([128, KT, 128], dt.float8e4, tag="wt8")
                for kb in range(KT // 4):
                    psD = p_pstw.tile([128, 8, 128, 2], dt.float8e4,
                                      tag="pstw")
                    ps = psD[:, 0:4]
                    for j in range(4):
                        kc = kb * 4 + j
                        nc.tensor.transpose(
                            out=ps[:, j, :, 0],
                            in_=q8[:, kc * 128:(kc + 1) * 128],
                            identity=ident8[:])
                    nc.scalar.copy(t[:, kb * 4:(kb + 1) * 4, :],
                                   ps[:, :, :, 0])
                return t

            def transpose_x8_pe(x8):
                """PE fp8 transposes of X8 -> [kp, kpair, 2, m] fp8 planes
                (canonical DoubleRow lhsT layout)."""
                t = p_x8T.tile([128, KD, 2, 128], dt.float8e4, tag="x8T")
                nkt = KT - KF0
                for kb in range(nkt // 4):
                    psD = p_pstw.tile([128, 8, 128, 2], dt.float8e4,
                                      tag="pstw")
                    for j in range(4):
                        kc = kb * 4 + j
                        nc.tensor.transpose(
                            out=psD[:, j, :, 0],
                            in_=x8[:, kc * 128:(kc + 1) * 128],
                            identity=ident8[:])
                    nc.scalar.copy(t[:, kb * 2:(kb + 1) * 2, :, :],
                                   psD[:, 0:4, :, 0])
                return t

            # ---------- prologue: weight quant + staged AllGather ----------
            wlA = wq_locA.ap().rearrange("p (kc n) -> p kc n", n=HALF_COLS)
            wlB = wq_locB.ap().rearrange("p (kc n) -> p kc n", n=HALF_COLS)
            def w_tile(wt):
                qw = quant_tile(w_sh, wt * 128, with_rms=False,
                                with_clamp=True)
                t = transpose_w_pe(qw)
                dst = wlA if wt < 2 else wlB
                c0 = (wt % 2) * 128
                nc.sync.dma_start(out=dst[:, :, c0:c0 + 128], in_=t[:])

            def fire_ag(loc, full):
                nc.gpsimd.collective_compute(
                    "AllGather", Alu.bypass,
                    replica_groups=[list(range(N_CORES))],
                    ins=[loc.ap().opt()], outs=[full.ap().opt()])

            # ---------- x quant ----------
            qxT = {}

            def quant_group(gi):
                for mt in range(gi * M_GROUP, (gi + 1) * M_GROUP):
                    q, x8 = quant_tile(x_sh, mt * 128, with_rms=True,
                                       with_clamp=False)
                    qxT[mt] = (transpose_q(q), transpose_x8_pe(x8))

            # ---------- GEMM ----------
            wfA = wq_fullA.ap().rearrange("(c p) (kc n) -> c p kc n",
                                          p=128, n=HALF_COLS)
            wfB = wq_fullB.ap().rearrange("(c p) (kc n) -> c p kc n",
                                          p=128, n=HALF_COLS)

            def chunk_defs():
                """(stage_aps, out_cols): two 256-col strips per chunk."""
                for j in range(4):
                    ca, cb = 2 * j, 2 * j + 1
                    yield ([wfA[ca], wfA[cb]],
                           [W_SHARD * ca, W_SHARD * cb])
                for j in range(4):
                    ca, cb = 2 * j, 2 * j + 1
                    yield ([wfB[ca], wfB[cb]],
                           [W_SHARD * ca + HALF_COLS,
                            W_SHARD * cb + HALF_COLS])

            def load_chunk(srcs_ap, eng=None):
                """One full-K tile per chunk, loaded with 2 DMAs: SWDGE
                issue cost (8 DMAs before) gated the first matmul after
                every AllGather / group boundary."""
                eng = eng or nc.gpsimd
                wt = p_qwT.tile([128, KT, W_SHARD], dt.float8e4, tag="qwT")
                eng.dma_start(out=wt[:, :, 0:HALF_COLS], in_=srcs_ap[0])
                eng.dma_start(out=wt[:, :, HALF_COLS:W_SHARD],
                              in_=srcs_ap[1])
                return wt

            def gemm_m(wk, out_cols, mt):
                ps = p_psum.tile([128, W_SHARD], dt.float32, tag="ps")
                tb, t8 = qxT[mt]
                for k in range(KF0):
                    nc.tensor.matmul(
                        ps[:], lhsT=tb[:, k, :], rhs=wk[:, k, :],
                        start=(k == 0), stop=False)
                for j in range(KD):
                    k = KF0 + 2 * j
                    nc.tensor.matmul(
                        ps[:], lhsT=t8[:, j, :, :], rhs=wk[:, k:k + 2, :],
                        start=False, stop=(j == KD - 1),
                        perf_mode=mybir.MatmulPerfMode.DoubleRow)
                st = p_stage.tile([128, W_SHARD], dt.bfloat16, tag="st")
                nc.scalar.activation(out=st[:], in_=ps[:], func=Act.Copy,
                                     scale=float(np.float32(OUT_SCALE)))
                for si, dst_col in enumerate(out_cols):
                    nc.sync.dma_start(
                        out=out[mt * 128:(mt + 1) * 128,
                                dst_col:dst_col + HALF_COLS],
                        in_=st[:, si * HALF_COLS:(si + 1) * HALF_COLS])

            def gemm_group(gi):
                for srcs_ap, out_cols in chunk_defs():
                    wk = load_chunk(srcs_ap, eng=nc.sync)
                    for mt in range(gi * M_GROUP, (gi + 1) * M_GROUP):
                        gemm_m(wk, out_cols, mt)

            # ----- group 0: interleave x-quant (PE transposes) with the
            # first chunk's per-m matmuls so the PE starts as soon as
            # AG-A lands and x0 is quantized -----
            chunks = list(chunk_defs())
            w_tile(0)
            w_tile(1)
            fire_ag(wq_locA, wq_fullA)
            w_tile(2)
            w_tile(3)
            fire_ag(wq_locB, wq_fullB)
            for mt in (0, 1):
                qm, x8m = quant_tile(x_sh, mt * 128, with_rms=True,
                                     with_clamp=False)
                qxT[mt] = (transpose_q_pe(qm), transpose_x8_pe(x8m))
            # A-phase of group 0, m-pair-major: each A-chunk is loaded,
            # used for an m-pair, reloaded for the next pair; B-chunk
            # loads ride the gpsimd (SWDGE) queue so their AG-B wait
            # cannot block the sync queue.
            for rep in range(2):
                if rep == 1:
                    for mt in (2, 3):
                        qm, x8m = quant_tile(x_sh, mt * 128, with_rms=True,
                                             with_clamp=False)
                        qxT[mt] = (transpose_q_pe(qm), transpose_x8_pe(x8m))
                for src_ap, out_cols in chunks[0:4]:
                    wk = load_chunk(src_ap)
                    for mt in (2 * rep, 2 * rep + 1):
                        gemm_m(wk, out_cols, mt)
            pend = {}

            def quant_only(mts):
                for mt in mts:
                    pend[mt] = quant_tile(x_sh, mt * 128, with_rms=True,
                                          with_clamp=False)

            def transpose_mt(mt):
                q, x8 = pend.pop(mt)
                qxT[mt] = (transpose_q(q), transpose_x8_pe(x8))

            def gemm_group2(gi, next_mts):
                nx = list(next_mts)
                for ci, (src_ap, out_cols) in enumerate(chunk_defs()):
                    wk = load_chunk(src_ap, eng=nc.sync)
                    for mt in range(gi * M_GROUP, (gi + 1) * M_GROUP):
                        gemm_m(wk, out_cols, mt)
                    if ci % 2 == 0 and nx:
                        transpose_mt(nx.pop(0))

            quant_only([4, 5, 6, 7])
            for ci, (src_ap, out_cols) in enumerate(chunks[4:]):
                wk = load_chunk(src_ap, eng=nc.gpsimd)
                for mt in range(4):
                    gemm_m(wk, out_cols, mt)
                transpose_mt(4 + ci)
            quant_only([8, 9, 10, 11])
            gemm_group2(1, [8, 9, 10, 11])
            quant_only([12, 13, 14, 15])
            gemm_group2(2, [12, 13, 14, 15])
            gemm_group2(3, [])

    nc.compile()
    return nc


_NC = None


def kernel(x, weight):
    global _NC
    if _NC is None:
        _NC = _build()
    x = np.ascontiguousarray(x)
    weight = np.ascontiguousarray(weight)
    in_maps = [
        {"x_sh": x[c * M_SHARD:(c + 1) * M_SHARD],
         "w_sh": weight[c * W_SHARD:(c + 1) * W_SHARD]}
        for c in range(N_CORES)
    ]
    res = run_bass_kernel_spmd(_NC, in_maps, list(range(N_CORES)))
    return np.concatenate(
        [np.asarray(res.results[c]["out"]).astype(np.float32)
         for c in range(N_CORES)], axis=0)



# revision 36
# speedup vs baseline: 1.0901x; 1.0307x over previous
"""NVFP4Linear (fused RMSNorm + NVFP4 quant-dequant + GEMM) on 8 TRN2 cores.

Final design (927us HW, rel err 0.0181 vs the 2e-2 gate, deterministic):
  - v2 skeleton: exact e4m3/e2m1 bit-trick quantization, [p,b,s]-contiguous
    quant passes, 2-stage fp8 AllGather of transposed quantized weights,
    512-col matmul chunks.
  - weights quantized straight to fp8: w8 = e4m3(qw*32).  Weight fp4*scale
    products almost never need a 4th mantissa bit (their scales are
    subnormal e4m3), so this costs only 2.5e-3 rel err and halves weight
    bytes end-to-end.  bf16 x fp8 matmuls stream at the same 263ns/512-col
    rate as bf16 x bf16 (PE moving-fetch is byte-rate-bound).
  - mixed-precision k-split (KF0=4): k-tiles [0,4) contract exactly
    (bf16 qx*16 x w8); k-tiles [4,32) use X8 = e4m3(qx*16) in k-paired fp8
    DoubleRow matmuls (256-deep contraction, 2x FLOP rate).  X8 rounding on
    7/8 of the k-sum adds sqrt(0.875)*0.0191 rel err; the qx*16 scaling is
    an exact power of two so both paths share one 1/512 output scale.
    (A full hi/lo-exact fp8 split was measured: DoubleRow with duplicated
    halves is byte-bound at the same speed as bf16 - exactness costs the
    entire 2x, so it was dropped for this calibrated partial split.)
  - X8 transposes ride the PE in fp8 step-2 psum mode (collective-immune);
    the bf16 k-slice uses xbar transposes, PE-mode for the first 4 tiles.
  - quant emitted ahead of transposes, transposes spread between GEMM
    chunks so PE never bunches up behind the DVE.
  - one SBUF tile per weight chunk, loaded with 2 DMAs (SWDGE issue cost
    gated the first matmul after every AllGather / group boundary).
"""

import sys

for _p in ("/opt/trn_rl_repo", "/root/.axon_site/_ro/trn_rl_repo"):
    if _p not in sys.path:
        sys.path.append(_p)

import numpy as np
import concourse.bass as bass  # noqa: F401
import concourse.mybir as mybir
import concourse.tile as tile
from concourse import bacc
from concourse.bass_utils import run_bass_kernel_spmd

dt = mybir.dt
Alu = mybir.AluOpType
Act = mybir.ActivationFunctionType

BLK = 16
EXP_MASK = 0x7F800000
F4_EXP_MIN = 0x3F800000
F4_H_ADD = 0x0B400000
E4M3_EXP_SUB = 0x01800000
E4M3_U_MIN = 0x3B000000
E4M3_H_ADD = 0x0BC00000
C8_MAX = E4M3_U_MIN + E4M3_EXP_SUB  # max first, then add
C8_ADD = E4M3_H_ADD - E4M3_EXP_SUB

N_CORES = 8
M_FULL, K, N = 16384, 4096, 4096
M_SHARD = M_FULL // N_CORES          # 2048
W_SHARD = N // N_CORES               # 512
KT = K // 128                        # 32
KT4 = KT // 4                        # 8 (k-quarter tiles for weight chunks)
KH = K // 2                          # 2048
M_TILES = M_SHARD // 128             # 16
M_GROUP = 4
G = M_TILES // M_GROUP               # 4
HALF_COLS = W_SHARD // 2             # 256
QC = 1024                            # ACT square quarter size
W_PRE = 32.0                         # qw prescale into e4m3 normal range
X_PRE = 16.0                         # qx prescale (exact power of two)
OUT_SCALE = 1.0 / (W_PRE * X_PRE)
KF0 = 2                              # k-tiles [0,KF0): exact bf16 path
KD = (KT - KF0) // 2                 # fp8 DoubleRow k-pairs


def _build():
    nc = bacc.Bacc("TRN2", target_bir_lowering=False, debug=False,
                   num_devices=N_CORES)

    x_sh = nc.declare_dram_parameter("x_sh", [M_SHARD, K], dt.bfloat16,
                                     isOutput=False)
    w_sh = nc.declare_dram_parameter("w_sh", [W_SHARD, K], dt.bfloat16,
                                     isOutput=False)
    # output stored bf16 on-device (halves store traffic); host widens to
    # fp32 — adds ~3e-4 RMS rounding, far inside the 2e-2 gate
    out = nc.declare_dram_parameter("out", [M_SHARD, N], dt.bfloat16,
                                    isOutput=True)

    # quantized transposed weight, layout (kp, kc, n):
    #   element = qw^T[k = kc*128+kp, shard col n]
    wq_locA = nc.dram_tensor("wq_locA", [128, KT * HALF_COLS], dt.float8e4)
    wq_locB = nc.dram_tensor("wq_locB", [128, KT * HALF_COLS], dt.float8e4)
    wq_fullA = nc.dram_tensor("wq_fullA", [N_CORES * 128, KT * HALF_COLS],
                              dt.float8e4, addr_space="Shared")
    wq_fullB = nc.dram_tensor("wq_fullB", [N_CORES * 128, KT * HALF_COLS],
                              dt.float8e4, addr_space="Shared")

    with tile.TileContext(nc) as tc:
        with (
            tc.tile_pool(name="src", bufs=4) as p_src,
            tc.tile_pool(name="f32", bufs=2) as p_f32,
            tc.tile_pool(name="q", bufs=4) as p_q,
            tc.tile_pool(name="q8w", bufs=2) as p_q8w,
            tc.tile_pool(name="wt8", bufs=2) as p_wt8,
            tc.tile_pool(name="sm", bufs=2) as p_sm,
            tc.tile_pool(name="row", bufs=4) as p_row,
            tc.tile_pool(name="qxT", bufs=2 * M_GROUP) as p_qxT,
            tc.tile_pool(name="x8", bufs=5) as p_x8,
            tc.tile_pool(name="x8T", bufs=2 * M_GROUP) as p_x8T,
            tc.tile_pool(name="qwT", bufs=2) as p_qwT,
            tc.tile_pool(name="stage", bufs=2) as p_stage,
            tc.tile_pool(name="psum", bufs=4, space="PSUM") as p_psum,
            tc.tile_pool(name="pst", bufs=1, space="PSUM") as p_pst,
            tc.tile_pool(name="pstw", bufs=2, space="PSUM") as p_pstw,
            tc.tile_pool(name="sqp", bufs=1, space="PSUM") as p_sqp,
        ):
            def quant_tile(dram_src, row0, with_rms, with_clamp):
                """Quantize 128 rows; returns q tile [128, K] bf16."""
                srcs = []
                for hi in range(2):
                    s = p_src.tile([128, KH], dt.bfloat16, tag="src")
                    nc.sync.dma_start(
                        out=s[:],
                        in_=dram_src[row0:row0 + 128, hi * KH:(hi + 1) * KH])
                    srcs.append(s)

                inv_rms_ap = None
                if with_rms:
                    ssum = p_row.tile([128, 8], dt.float32, tag="ssum")
                    for qi in range(8):
                        sq = p_sqp.tile([128, 512], dt.float32, tag="sq")
                        sh = srcs[qi // 4]
                        o = (qi % 4) * 512
                        nc.scalar.activation(
                            out=sq[:], in_=sh[:, o:o + 512],
                            func=Act.Square, accum_out=ssum[:, qi:qi + 1])
                    ssum2 = p_row.tile([128, 1], dt.float32, tag="ssum2")
                    nc.vector.tensor_reduce(
                        out=ssum2[:], in_=ssum[:],
                        axis=mybir.AxisListType.X, op=Alu.add)
                    ms = p_row.tile([128, 1], dt.float32, tag="ms")
                    nc.vector.tensor_scalar(
                        out=ms[:], in0=ssum2[:],
                        scalar1=float(np.float32(1.0 / K)), scalar2=1e-6,
                        op0=Alu.mult, op1=Alu.add)
                    srms = p_row.tile([128, 1], dt.float32, tag="srms")
                    nc.scalar.activation(out=srms[:], in_=ms[:],
                                         func=Act.Sqrt)
                    invr = p_row.tile([128, 1], dt.float32, tag="invr")
                    nc.vector.reciprocal(invr[:], srms[:])
                    inv_rms_ap = invr[:]

                # ---- per-block scales (tile-wide, [128, 256]) ----
                nb = K // BLK
                nbh = nb // 2
                amax = p_sm.tile([128, nb], dt.float32, tag="amax")
                for hi in range(2):
                    nc.vector.tensor_reduce(
                        out=amax[:, hi * nbh:(hi + 1) * nbh],
                        in_=srcs[hi][:].rearrange("p (b s) -> p b s", s=BLK),
                        axis=mybir.AxisListType.X,
                        op=Alu.max, apply_absolute_value=True)

                v = p_sm.tile([128, nb], dt.float32, tag="v")
                if inv_rms_ap is not None:
                    nc.vector.tensor_scalar(
                        out=v[:], in0=amax[:], scalar1=inv_rms_ap,
                        scalar2=float(np.float32(1.0 / 6.0)),
                        op0=Alu.mult, op1=Alu.mult)
                else:
                    nc.vector.tensor_scalar(
                        out=v[:], in0=amax[:],
                        scalar1=float(np.float32(1.0 / 6.0)), scalar2=None,
                        op0=Alu.mult)

                h8 = p_sm.tile([128, nb], dt.float32, tag="h8")
                nc.vector.tensor_scalar(
                    out=h8[:].bitcast(dt.int32), in0=v[:].bitcast(dt.int32),
                    scalar1=EXP_MASK, scalar2=None, op0=Alu.bitwise_and)
                nc.vector.tensor_scalar(
                    out=h8[:].bitcast(dt.int32), in0=h8[:].bitcast(dt.int32),
                    scalar1=C8_MAX, scalar2=C8_ADD, op0=Alu.max, op1=Alu.add)
                scal = p_sm.tile([128, nb], dt.float32, tag="scal")
                nc.vector.tensor_tensor(out=scal[:], in0=v[:], in1=h8[:],
                                        op=Alu.add)
                nc.vector.tensor_tensor(out=scal[:], in0=scal[:], in1=h8[:],
                                        op=Alu.subtract)
                g = p_sm.tile([128, nb], dt.float32, tag="g")
                nc.vector.reciprocal(g[:], scal[:])
                if inv_rms_ap is not None:
                    nc.vector.tensor_scalar(
                        out=g[:], in0=g[:], scalar1=inv_rms_ap,
                        scalar2=1.0e30, op0=Alu.mult, op1=Alu.min)
                else:
                    # weight path: fold ties-up fudge (1+2^-23) into g
                    nc.vector.tensor_scalar(
                        out=g[:], in0=g[:],
                        scalar1=float(np.float32(1.0 + 2.0 ** -23)),
                        scalar2=1.0e30, op0=Alu.mult, op1=Alu.min)

                if with_rms:
                    q = p_q.tile([128, K], dt.bfloat16, tag="q")
                else:
                    q = p_q8w.tile([128, K], dt.float8e4, tag="q8w")

                # ---- big passes, per half ----
                for hi in range(2):
                    bsl = slice(hi * nbh, (hi + 1) * nbh)
                    ax = p_f32.tile([128, KH], dt.float32, tag="ax")
                    hc = p_f32.tile([128, KH], dt.float32, tag="hc")
                    ax_bs = ax[:].rearrange("p (b s) -> p b s", s=BLK)
                    g_b = g[:, bsl, None].broadcast_to([128, nbh, BLK])
                    src_h = srcs[hi][:].rearrange("p (b s) -> p b s", s=BLK)
                    # P1: ax = src * g
                    nc.vector.tensor_tensor(out=ax_bs, in0=src_h, in1=g_b,
                                            op=Alu.mult)
                    if with_clamp:
                        nc.vector.tensor_scalar(
                            out=ax[:], in0=ax[:], scalar1=6.0, scalar2=-6.0,
                            op0=Alu.min, op1=Alu.max)
                    # P2: hc = ax & EXP_MASK
                    nc.vector.tensor_scalar(
                        out=hc[:].bitcast(dt.int32),
                        in0=ax[:].bitcast(dt.int32),
                        scalar1=EXP_MASK, scalar2=None, op0=Alu.bitwise_and)
                    # P3: hc = (hc max F4_EXP_MIN) + F4_H_ADD
                    nc.vector.tensor_scalar(
                        out=hc[:].bitcast(dt.int32),
                        in0=hc[:].bitcast(dt.int32),
                        scalar1=F4_EXP_MIN, scalar2=F4_H_ADD,
                        op0=Alu.max, op1=Alu.add)
                    # P4/P5: ax = (ax + hc) - hc = round(ax)  (in-place, 2x)
                    nc.vector.tensor_tensor(out=ax[:], in0=ax[:], in1=hc[:],
                                            op=Alu.add)
                    nc.vector.tensor_tensor(out=ax[:], in0=ax[:], in1=hc[:],
                                            op=Alu.subtract)
                    # P6: q = ax * scal  (x: bf16; w: e4m3(q * 32))
                    q_bs = q[:, hi * KH:(hi + 1) * KH].rearrange(
                        "p (b s) -> p b s", s=BLK)
                    scal_b = scal[:, bsl, None].broadcast_to([128, nbh, BLK])
                    pre = X_PRE if with_rms else W_PRE
                    nc.vector.scalar_tensor_tensor(
                        out=q_bs, in0=ax_bs, scalar=pre, in1=scal_b,
                        op0=Alu.mult, op1=Alu.mult)
                if not with_rms:
                    return q
                # X8 = e4m3(qx*16) for the DoubleRow k-half (RTNE; q holds
                # qx*16 exactly in bf16, so this matches the f32 rounding)
                x8 = p_x8.tile([128, K - KF0 * 128], dt.float8e4, tag="x8")
                nc.scalar.activation(out=x8[:], in_=q[:, KF0 * 128:K],
                                     func=Act.Copy)
                return q, x8

            def transpose_q(q):
                """xbar transpose of the exact-bf16 k-half — stalls while a
                collective is in flight."""
                t = p_qxT.tile([128, KF0, 128], dt.bfloat16, tag="qxT")
                nc.sync.dma_start_transpose(out=t[:], in_=q[:, 0:KF0 * 128])
                return t

            # identity for PE-mode transpose (collective-immune)
            ones = p_row.tile([128, 128], dt.bfloat16, tag="ones")
            nc.vector.memset(ones[:], 1.0)
            ident = p_row.tile([128, 128], dt.bfloat16, tag="ident")
            nc.gpsimd.affine_select(
                out=ident[:], in_=ones[:], pattern=[[-1, 128]],
                compare_op=Alu.is_equal, fill=0.0,
                base=0, channel_multiplier=1)

            def transpose_q_pe(q):
                """Transpose on the (idle) PE + ACT copies, no xbar.
                4 k-slices batched per PSUM bank -> 8 ACT copies."""
                t = p_qxT.tile([128, KF0, 128], dt.bfloat16, tag="qxT")
                for kb in range((KF0 + 3) // 4):
                    nblk = min(4, KF0 - kb * 4)
                    ps = p_pst.tile([128, 4, 128], dt.bfloat16, tag="pst")
                    for j in range(nblk):
                        kc = kb * 4 + j
                        nc.tensor.transpose(
                            out=ps[:, j, :],
                            in_=q[:, kc * 128:(kc + 1) * 128],
                            identity=ident[:])
                    nc.scalar.copy(t[:, kb * 4:kb * 4 + nblk, :],
                                   ps[:, 0:nblk, :])
                return t

            ones8 = p_row.tile([128, 128], dt.float8e4, tag="ones8")
            nc.vector.memset(ones8[:], 1.0)
            ident8 = p_row.tile([128, 128], dt.float8e4, tag="ident8")
            nc.gpsimd.affine_select(
                out=ident8[:], in_=ones8[:], pattern=[[-1, 128]],
                compare_op=Alu.is_equal, fill=0.0,
                base=0, channel_multiplier=1)

            def transpose_w_pe(q8):
                """PE fp8 transpose (step-2 psum) -> [kp, kc, n] fp8."""
                t = p_wt8.tile([128, KT, 128], dt.float8e4, tag="wt8")
                for kb in range(KT // 4):
                    psD = p_pstw.tile([128, 8, 128, 2], dt.float8e4,
                                      tag="pstw")
                    ps = psD[:, 0:4]
                    for j in range(4):
                        kc = kb * 4 + j
                        nc.tensor.transpose(
                            out=ps[:, j, :, 0],
                            in_=q8[:, kc * 128:(kc + 1) * 128],
                            identity=ident8[:])
                    nc.scalar.copy(t[:, kb * 4:(kb + 1) * 4, :],
                                   ps[:, :, :, 0])
                return t

            def transpose_x8_pe(x8):
                """PE fp8 transposes of X8 -> [kp, kpair, 2, m] fp8 planes
                (canonical DoubleRow lhsT layout)."""
                t = p_x8T.tile([128, KD, 2, 128], dt.float8e4, tag="x8T")
                nkt = KT - KF0
                for kb in range((nkt + 3) // 4):
                    nblk = min(4, nkt - kb * 4)
                    psD = p_pstw.tile([128, 8, 128, 2], dt.float8e4,
                                      tag="pstw")
                    for j in range(nblk):
                        kc = kb * 4 + j
                        nc.tensor.transpose(
                            out=psD[:, j, :, 0],
                            in_=x8[:, kc * 128:(kc + 1) * 128],
                            identity=ident8[:])
                    nc.scalar.copy(t[:, kb * 2:kb * 2 + nblk // 2, :, :],
                                   psD[:, 0:nblk, :, 0])
                return t

            # ---------- prologue: weight quant + staged AllGather ----------
            wlA = wq_locA.ap().rearrange("p (kc n) -> p kc n", n=HALF_COLS)
            wlB = wq_locB.ap().rearrange("p (kc n) -> p kc n", n=HALF_COLS)
            def w_tile(wt):
                qw = quant_tile(w_sh, wt * 128, with_rms=False,
                                with_clamp=True)
                t = transpose_w_pe(qw)
                dst = wlA if wt < 2 else wlB
                c0 = (wt % 2) * 128
                nc.sync.dma_start(out=dst[:, :, c0:c0 + 128], in_=t[:])

            def fire_ag(loc, full):
                nc.gpsimd.collective_compute(
                    "AllGather", Alu.bypass,
                    replica_groups=[list(range(N_CORES))],
                    ins=[loc.ap().opt()], outs=[full.ap().opt()])

            # ---------- x quant ----------
            qxT = {}

            def quant_group(gi):
                for mt in range(gi * M_GROUP, (gi + 1) * M_GROUP):
                    q, x8 = quant_tile(x_sh, mt * 128, with_rms=True,
                                       with_clamp=False)
                    qxT[mt] = (transpose_q(q), transpose_x8_pe(x8))

            # ---------- GEMM ----------
            wfA = wq_fullA.ap().rearrange("(c p) (kc n) -> c p kc n",
                                          p=128, n=HALF_COLS)
            wfB = wq_fullB.ap().rearrange("(c p) (kc n) -> c p kc n",
                                          p=128, n=HALF_COLS)

            def chunk_defs():
                """(stage_aps, out_cols): two 256-col strips per chunk."""
                for j in range(4):
                    ca, cb = 2 * j, 2 * j + 1
                    yield ([wfA[ca], wfA[cb]],
                           [W_SHARD * ca, W_SHARD * cb])
                for j in range(4):
                    ca, cb = 2 * j, 2 * j + 1
                    yield ([wfB[ca], wfB[cb]],
                           [W_SHARD * ca + HALF_COLS,
                            W_SHARD * cb + HALF_COLS])

            def load_chunk(srcs_ap, eng=None):
                """One full-K tile per chunk, loaded with 2 DMAs: SWDGE
                issue cost (8 DMAs before) gated the first matmul after
                every AllGather / group boundary."""
                eng = eng or nc.gpsimd
                wt = p_qwT.tile([128, KT, W_SHARD], dt.float8e4, tag="qwT")
                eng.dma_start(out=wt[:, :, 0:HALF_COLS], in_=srcs_ap[0])
                eng.dma_start(out=wt[:, :, HALF_COLS:W_SHARD],
                              in_=srcs_ap[1])
                return wt

            def gemm_m(wk, out_cols, mt):
                ps = p_psum.tile([128, W_SHARD], dt.float32, tag="ps")
                tb, t8 = qxT[mt]
                for k in range(KF0):
                    nc.tensor.matmul(
                        ps[:], lhsT=tb[:, k, :], rhs=wk[:, k, :],
                        start=(k == 0), stop=False)
                for j in range(KD):
                    k = KF0 + 2 * j
                    nc.tensor.matmul(
                        ps[:], lhsT=t8[:, j, :, :], rhs=wk[:, k:k + 2, :],
                        start=False, stop=(j == KD - 1),
                        perf_mode=mybir.MatmulPerfMode.DoubleRow)
                st = p_stage.tile([128, W_SHARD], dt.bfloat16, tag="st")
                nc.scalar.activation(out=st[:], in_=ps[:], func=Act.Copy,
                                     scale=float(np.float32(OUT_SCALE)))
                for si, dst_col in enumerate(out_cols):
                    nc.sync.dma_start(
                        out=out[mt * 128:(mt + 1) * 128,
                                dst_col:dst_col + HALF_COLS],
                        in_=st[:, si * HALF_COLS:(si + 1) * HALF_COLS])

            def gemm_group(gi):
                for srcs_ap, out_cols in chunk_defs():
                    wk = load_chunk(srcs_ap, eng=nc.sync)
                    for mt in range(gi * M_GROUP, (gi + 1) * M_GROUP):
                        gemm_m(wk, out_cols, mt)

            # ----- group 0: interleave x-quant (PE transposes) with the
            # first chunk's per-m matmuls so the PE starts as soon as
            # AG-A lands and x0 is quantized -----
            chunks = list(chunk_defs())
            w_tile(0)
            w_tile(1)
            fire_ag(wq_locA, wq_fullA)
            w_tile(2)
            w_tile(3)
            fire_ag(wq_locB, wq_fullB)
            for mt in (0, 1):
                qm, x8m = quant_tile(x_sh, mt * 128, with_rms=True,
                                     with_clamp=False)
                qxT[mt] = (transpose_q_pe(qm), transpose_x8_pe(x8m))
            # A-phase of group 0, m-pair-major: each A-chunk is loaded,
            # used for an m-pair, reloaded for the next pair; B-chunk
            # loads ride the gpsimd (SWDGE) queue so their AG-B wait
            # cannot block the sync queue.
            for rep in range(2):
                if rep == 1:
                    for mt in (2, 3):
                        qm, x8m = quant_tile(x_sh, mt * 128, with_rms=True,
                                             with_clamp=False)
                        qxT[mt] = (transpose_q_pe(qm), transpose_x8_pe(x8m))
                for src_ap, out_cols in chunks[0:4]:
                    wk = load_chunk(src_ap)
                    for mt in (2 * rep, 2 * rep + 1):
                        gemm_m(wk, out_cols, mt)
            pend = {}

            def quant_only(mts):
                for mt in mts:
                    pend[mt] = quant_tile(x_sh, mt * 128, with_rms=True,
                                          with_clamp=False)

            def transpose_mt(mt):
                q, x8 = pend.pop(mt)
                qxT[mt] = (transpose_q(q), transpose_x8_pe(x8))

            def gemm_group2(gi, next_mts):
                nx = list(next_mts)
                for ci, (src_ap, out_cols) in enumerate(chunk_defs()):
                    wk = load_chunk(src_ap, eng=nc.sync)
                    for mt in range(gi * M_GROUP, (gi + 1) * M_GROUP):
                        gemm_m(wk, out_cols, mt)
                    if ci % 2 == 0 and nx:
                        transpose_mt(nx.pop(0))

            quant_only([4, 5, 6, 7])
            for ci, (src_ap, out_cols) in enumerate(chunks[4:]):
                wk = load_chunk(src_ap, eng=nc.gpsimd)
                for mt in range(4):
                    gemm_m(wk, out_cols, mt)
                transpose_mt(4 + ci)
            quant_only([8, 9, 10, 11])
            gemm_group2(1, [8, 9, 10, 11])
            quant_only([12, 13, 14, 15])
            gemm_group2(2, [12, 13, 14, 15])
            gemm_group2(3, [])

    nc.compile()
    return nc


_NC = None


def kernel(x, weight):
    global _NC
    if _NC is None:
        _NC = _build()
    x = np.ascontiguousarray(x)
    weight = np.ascontiguousarray(weight)
    in_maps = [
        {"x_sh": x[c * M_SHARD:(c + 1) * M_SHARD],
         "w_sh": weight[c * W_SHARD:(c + 1) * W_SHARD]}
        for c in range(N_CORES)
    ]
    res = run_bass_kernel_spmd(_NC, in_maps, list(range(N_CORES)))
    return np.concatenate(
        [np.asarray(res.results[c]["out"]).astype(np.float32)
         for c in range(N_CORES)], axis=0)

